# revision 13
# baseline (speedup 1.0000x reference)
"""Trainium2 Bass kernel for nn_DVT_69475390980615 (dense_transformer).

Sharding: 8 cores = 4 batches x 2 head-halves. Core c handles batch c//2
and heads [4*(c%2), 4*(c%2)+4).

Host-side folds (numpy, cheap):
  - BN scale folded into conv weights; SCALE folded into Wq.
  - Branch-1 (learned position logits): A = diag(gmk) @ Wmk @ diag(gq) @ Wq
    so dots1^T = (A @ x)^T computed directly from x (contraction over the
    256 input channels) -- q1 never materialized for branch 1.
  - Relative-position bias pos_emb[idx]/SCALE + per-row c1 precomputed as
    fp16 (4,1024,1024) per core; DMA-cast to f32 on device.
  - All weights pre-transposed into [K, M] (lhsT) layouts.

Device dataflow (per core, all f32):
  - dots computed transposed [j-part, i-free]; softmax denominator comes
    free from a ones-column appended to V^T in the P@V matmul (M=65);
    no max subtraction (logits bounded ~+-35, exp safe in f32).
  - exp on ACT straight out of PSUM (branch 2) or after the DVE bias add
    (branch 1); normalization applied to the small [64,1024] PV output.
  - gelu(concat(out1,out2)) -> partial Wo matmul over this core's 512
    channels. Host sums the two half partials per batch and applies the
    final affine (bo, go, bo2).
"""

import sys

for _p in ("/opt/trn_rl_repo",):
    if _p not in sys.path:
        sys.path.insert(0, _p)

import numpy as np

FMAP = 32
HEADS = 8
DK = 32
DV = 64
N = FMAP * FMAP  # 1024
DIM = 256
B = 4
SCALE = DK ** -0.5
HPC = 4  # heads per core
N_CORES = 8

_PROGRAM = None


def _pos_idx():
    r = np.arange(FMAP)
    ii, jj = np.meshgrid(r, r, indexing="ij")
    pos = np.stack([ii.reshape(-1), jj.reshape(-1)], axis=-1)  # (N,2)
    rel = np.abs(pos[:, None, :] - pos[None, :, :])  # (N,N,2)
    return rel[..., 0] * FMAP + rel[..., 1]  # (N,N) int


def _build_program():
    import concourse.bass as bass
    import concourse.tile as tile
    from concourse import bacc, mybir
    from concourse.bass import ts
    from concourse.masks import make_identity
    from contextlib import ExitStack

    f32 = mybir.dt.float32
    f16 = mybir.dt.float16
    f32r = mybir.dt.float32r
    AF = mybir.ActivationFunctionType

    def r(ap):
        return ap

    nc = bacc.Bacc(
        "TRN2",
        target_bir_lowering=False,
        debug=False,
        enable_asserts=False,
        num_devices=N_CORES,
    )

    x_d = nc.dram_tensor("x", [DIM, N], f32r, kind="ExternalInput").ap()
    at_d = nc.dram_tensor("at", [DIM, HPC * N], f32r, kind="ExternalInput").ap()
    bias_d = nc.dram_tensor("bias", [HPC, N, N], f32, kind="ExternalInput").ap()
    wqt_d = nc.dram_tensor("wqt", [DIM, HPC * DK], f32r, kind="ExternalInput").ap()
    wkt_d = nc.dram_tensor("wkt", [DIM, HPC * DK], f32r, kind="ExternalInput").ap()
    wvt_d = nc.dram_tensor("wvt", [DIM, HPC * DV], f32r, kind="ExternalInput").ap()
    bq_d = nc.dram_tensor("bq", [HPC * DK, 1], f32, kind="ExternalInput").ap()
    bk_d = nc.dram_tensor("bk", [HPC * DK, 1], f32, kind="ExternalInput").ap()
    bv_d = nc.dram_tensor("bv", [2, 128, 1], f32, kind="ExternalInput").ap()
    wot_d = nc.dram_tensor("wot", [2 * HPC, DV, DIM], f32, kind="ExternalInput").ap()
    out_d = nc.dram_tensor("out", [DIM, N], f32, kind="ExternalOutput").ap()

    with tile.TileContext(nc) as tc, ExitStack() as ctx:
        const = ctx.enter_context(tc.tile_pool(name="const", bufs=1))

        # ---- persistent loads ----
        xs = const.tile([128, 2, N], f32r)
        nc.sync.dma_start(xs[:], x_d.rearrange("(t p) i -> p t i", p=128))
        wqts = const.tile([128, 2, HPC * DK], f32r)
        nc.sync.dma_start(wqts[:], wqt_d.rearrange("(t p) m -> p t m", p=128))
        wkts = const.tile([128, 2, HPC * DK], f32r)
        nc.sync.dma_start(wkts[:], wkt_d.rearrange("(t p) m -> p t m", p=128))
        wvts = const.tile([128, 2, HPC * DV], f32r)
        nc.sync.dma_start(wvts[:], wvt_d.rearrange("(t p) m -> p t m", p=128))
        wots = const.tile([64, 2 * HPC, DIM], f32)
        nc.sync.dma_start(wots[:], wot_d.rearrange("g p o -> p g o"))
        bqs = const.tile([128, 1], f32)
        nc.sync.dma_start(bqs[:], bq_d)
        bks = const.tile([128, 1], f32)
        nc.sync.dma_start(bks[:], bk_d)
        bvs = const.tile([128, 2, 1], f32)
        nc.sync.dma_start(bvs[:], bv_d.rearrange("t p one -> p t one"))
        ident = const.tile([128, 128], f32)
        make_identity(nc, ident[:])

        qs = const.tile([128, N], f32r)
        ks = const.tile([128, N], f32r)
        vs = const.tile([128, 2, N], f32)
        vts = const.tile([128, 8, HPC * (DV + 1)], f32r)  # [j, jt, (h, d|ones)]
        gb = const.tile([64, 2 * HPC, N], f32)  # gelu input, rows=d, dim1=(br,h)
        gb2 = const.tile([64, 2 * HPC, N], f32)  # gelu output
        ob = const.tile([128, 2, N], f32)  # final partial output

        # ---- phase 1: Q, K, V projections ----
        with tc.tile_pool(name="p1", bufs=2, space="PSUM") as p1:
            for isl in range(2):
                pq = p1.tile([128, 512], f32, tag="pqkv")
                for kt in range(2):
                    nc.tensor.matmul(
                        pq[:], r(wqts[:, kt]), r(xs[:, kt, ts(isl, 512)]),
                        start=kt == 0, stop=kt == 1,
                    )
                nc.vector.tensor_scalar_add(qs[:, ts(isl, 512)], pq[:], bqs[:])
                pk = p1.tile([128, 512], f32, tag="pqkv")
                for kt in range(2):
                    nc.tensor.matmul(
                        pk[:], r(wkts[:, kt]), r(xs[:, kt, ts(isl, 512)]),
                        start=kt == 0, stop=kt == 1,
                    )
                nc.vector.tensor_scalar_add(ks[:, ts(isl, 512)], pk[:], bks[:])
                for mt in range(2):
                    pv = p1.tile([128, 512], f32, tag="pqkv")
                    for kt in range(2):
                        nc.tensor.matmul(
                            pv[:], r(wvts[:, kt, ts(mt, 128)]), r(xs[:, kt, ts(isl, 512)]),
                            start=kt == 0, stop=kt == 1,
                        )
                    nc.vector.tensor_scalar_add(
                        vs[:, mt, ts(isl, 512)], pv[:], bvs[:, mt]
                    )

        # ---- phase 2: V^T with ones columns ----
        ones_s = const.tile([128, 8, 1], f32)
        nc.vector.memset(ones_s[:], 1.0)
        for h in range(HPC):
            nc.vector.tensor_copy(out=vts[:, :, h * (DV + 1) + DV], in_=ones_s[:])
        with tc.tile_pool(name="p2", bufs=2, space="PSUM") as p2:
            for jt in range(8):
                for dtile in range(2):
                    pt = p2.tile([128, 128], f32, tag="ptr")
                    nc.tensor.transpose(pt[:], vs[:, dtile, ts(jt, 128)], ident[:])
                    for sub in range(2):
                        h = dtile * 2 + sub
                        nc.any.tensor_copy(
                            out=vts[:, jt, h * (DV + 1) : h * (DV + 1) + DV],
                            in_=pt[:, ts(sub, DV)],
                        )

        # ---- phase 3: attention, both branches ----
        with (
            tc.tile_pool(name="pd", bufs=2, space="PSUM") as pdp,
            tc.tile_pool(name="po", bufs=4, space="PSUM") as pop,
            tc.tile_pool(name="apool", bufs=3) as apool,
            tc.tile_pool(name="bpool", bufs=3) as bpool,
            tc.tile_pool(name="prepool", bufs=2) as prepool,
            tc.tile_pool(name="atile", bufs=2) as atile_pool,
            tc.tile_pool(name="small", bufs=2) as small,
            tc.tile_pool(name="drsc", bufs=4, space="DRAM") as drsc,
        ):
            for h in range(HPC):
                ath = atile_pool.tile([128, 2, N], f32r, tag="ath")
                nc.sync.dma_start(
                    ath[:],
                    at_d.rearrange("(t p) j -> p t j", p=128)[
                        :, :, h * N : (h + 1) * N
                    ],
                )
                for br in (1, 0):
                    po = [
                        pop.tile([128, 512], f32, tag="po", name=f"po_{i}")
                        for i in range(2)
                    ]
                    for jt in range(8):
                        attn = apool.tile([128, N], f32r, tag="attn")
                        if br == 0:
                            bt = bpool.tile([128, N], f32, tag="bt")
                            nc.sync.dma_start(bt[:], bias_d[h, ts(jt, 128), :])
                            pd = pdp.tile([128, N], f32, tag="pd")
                            for isl in range(2):
                                for kt in range(2):
                                    nc.tensor.matmul(
                                        pd[:, ts(isl, 512)],
                                        r(ath[:, kt, ts(jt, 128)]),
                                        r(xs[:, kt, ts(isl, 512)]),
                                        start=kt == 0, stop=kt == 1,
                                    )
                            pre = prepool.tile([128, N], f32, tag="pre")
                            nc.vector.tensor_add(pre[:], pd[:], bt[:])
                            nc.scalar.activation(attn[:], pre[:], AF.Exp)
                        else:
                            pd = pdp.tile([128, N], f32, tag="pd")
                            for isl in range(2):
                                nc.tensor.matmul(
                                    pd[:, ts(isl, 512)],
                                    r(ks[h * DK : (h + 1) * DK, ts(jt, 128)]),
                                    r(qs[h * DK : (h + 1) * DK, ts(isl, 512)]),
                                    start=True, stop=True,
                                    tile_position=(h * DK, 0),
                                )
                            nc.scalar.activation(attn[:], pd[:], AF.Exp)
                        for isl in range(2):
                            nc.tensor.matmul(
                                po[isl][0 : DV + 1, :],
                                r(vts[:, jt, h * (DV + 1) : (h + 1) * (DV + 1)]),
                                r(attn[:, ts(isl, 512)]),
                                start=jt == 0, stop=jt == 7,
                            )
                    hb = br * HPC + h
                    ub = small.tile([128, 2, 512], f32, tag="ub")
                    for isl in range(2):
                        nc.vector.tensor_copy(
                            out=ub[0 : DV + 1, isl], in_=po[isl][0 : DV + 1, :]
                        )
                    for isl in range(2):
                        rtmp = small.tile([128, 512], f32, tag="rtmp")
                        nc.vector.reciprocal(
                            rtmp[DV : DV + 1, :], ub[DV : DV + 1, isl]
                        )
                        sc = drsc.tile([1, 512], f32, tag="sc")
                        nc.sync.dma_start(sc[:], rtmp[DV : DV + 1, :])
                        rb = small.tile([DV, 512], f32, tag="rb")
                        nc.sync.dma_start(rb[:], sc[:].to_broadcast((DV, 512)))
                        nc.vector.tensor_mul(
                            gb[:, hb, ts(isl, 512)], ub[0:DV, isl], rb[:]
                        )

        # ---- phase 4: gelu ----
        nc.scalar.activation(gb2[:], gb[:], AF.Gelu)

        # ---- phase 5: partial Wo ----
        with tc.tile_pool(name="p5", bufs=2, space="PSUM") as p5:
            for ot in range(2):
                for isl in range(2):
                    pw = p5.tile([128, 512], f32, tag="pw")
                    for kt in range(2 * HPC):
                        nc.tensor.matmul(
                            pw[:],
                            r(wots[:, kt, ts(ot, 128)]),
                            r(gb2[:, kt, ts(isl, 512)]),
                            start=kt == 0, stop=kt == 2 * HPC - 1,
                        )
                    nc.any.tensor_copy(ob[:, ot, ts(isl, 512)], pw[:])
        nc.sync.dma_start(out_d.rearrange("(t p) i -> p t i", p=128), ob[:])

    nc.compile()
    return nc


def _prepare_in_maps(inputs):
    x = np.asarray(inputs["x"], np.float32)
    Wq = np.asarray(inputs["Wq"], np.float32)
    gq = np.asarray(inputs["gq"], np.float32)
    bq = np.asarray(inputs["bq"], np.float32)
    Wk = np.asarray(inputs["Wk"], np.float32)
    gk = np.asarray(inputs["gk"], np.float32)
    bk = np.asarray(inputs["bk"], np.float32)
    Wv = np.asarray(inputs["Wv"], np.float32)
    gv = np.asarray(inputs["gv"], np.float32)
    bv = np.asarray(inputs["bv"], np.float32)
    Wmk = np.asarray(inputs["Wmk"], np.float32)
    gmk = np.asarray(inputs["gmk"], np.float32)
    bmk = np.asarray(inputs["bmk"], np.float32)
    pos_emb = np.asarray(inputs["pos_emb"], np.float32)
    Wo = np.asarray(inputs["Wo"], np.float32)

    # BN folds
    Wq_f = gq[:, None] * Wq            # unscaled (for branch 1 fold)
    Wq_s = Wq_f * SCALE                # scaled (branch 2 q)
    bq_s = bq * SCALE
    Wk_f = gk[:, None] * Wk
    Wv_f = gv[:, None] * Wv

    # branch-1 fused matrix and per-row constant
    A = (gmk[:, None] * Wmk) @ Wq_f    # (H*N, DIM)
    c1 = gmk * (Wmk @ bq) + bmk        # (H*N,)

    # full position bias per head: B[h, j, i] = pos_emb[idx[j,i],h]/SCALE + c1[h*N+j]
    idx = _pos_idx()
    Ball = pos_emb[idx] / SCALE        # (N, N, H)  [symmetric in (i,j)]
    Ball = np.ascontiguousarray(np.transpose(Ball, (2, 0, 1)))  # (H, j, i)
    Ball += c1.reshape(HEADS, N, 1)
    Ball32 = np.ascontiguousarray(Ball, np.float32)

    x2 = x.reshape(B, DIM, N)

    in_maps = []
    for core in range(N_CORES):
        b = core // 2
        half = core % 2
        hs = half * HPC
        qrows = slice(hs * DK, (hs + HPC) * DK)
        vrows = slice(hs * DV, (hs + HPC) * DV)
        arows = slice(hs * N, (hs + HPC) * N)

        wot = np.empty((2 * HPC, DV, DIM), np.float32)
        for g in range(2 * HPC):
            br, h = divmod(g, HPC)
            c0 = br * HEADS * DV + (hs + h) * DV
            wot[g] = Wo[:, c0 : c0 + DV].T

        in_maps.append({
            "x": np.ascontiguousarray(x2[b]),
            "at": np.ascontiguousarray(A[arows].T),
            "bias": Ball32[hs : hs + HPC],
            "wqt": np.ascontiguousarray(Wq_s[qrows].T),
            "wkt": np.ascontiguousarray(Wk_f[qrows].T),
            "wvt": np.ascontiguousarray(Wv_f[vrows].T),
            "bq": np.ascontiguousarray(bq_s[qrows].reshape(-1, 1)),
            "bk": np.ascontiguousarray(bk[qrows].reshape(-1, 1)),
            "bv": np.ascontiguousarray(bv[vrows].reshape(2, 128, 1)),
            "wot": wot,
        })
    return in_maps


def get_program():
    global _PROGRAM
    if _PROGRAM is None:
        _PROGRAM = _build_program()
    return _PROGRAM


def run_cores(inputs, **run_kwargs):
    """Compile/run the SPMD program; returns (BassKernelResults, in_maps)."""
    from concourse.bass_utils import run_bass_kernel_spmd

    nc = get_program()
    in_maps = _prepare_in_maps(inputs)
    res = run_bass_kernel_spmd(
        nc, in_maps, core_ids=list(range(N_CORES)), **run_kwargs
    )
    return res


def kernel(**inputs):
    bo = np.asarray(inputs["bo"], np.float32)
    go = np.asarray(inputs["go"], np.float32)
    bo2 = np.asarray(inputs["bo2"], np.float32)

    res = run_cores(inputs)

    out = np.empty((B, DIM, N), np.float32)
    cbias = (bo * go + bo2)[:, None]
    for b in range(B):
        p = res.results[2 * b]["out"] + res.results[2 * b + 1]["out"]
        out[b] = p * go[:, None] + cbias
    return out.reshape(B, DIM, FMAP, FMAP)
